# revision 5
# baseline (speedup 1.0000x reference)
"""Dimension-adaptive max pooling for sensors — Trainium2 Bass kernel.

Problem: x (64, 512, 48, 64) f32 -> out (64, 16*6*64) = (64, 6144) f32.
Adaptive max pool over spatial dims (512, 48) into (16, 6) bins. Since
512/16 = 32 and 48/6 = 8 exactly, each output bin is a plain max over a
(32, 8) window:

  out[b, iw*384 + ih*64 + m] = max_{r<32, hh<8} x[b, iw*32+r, ih*8+hh, m]

Sharding: pure data parallel over batch. 8 cores x 8 samples each.

Per-core layout: x[b] is a contiguous (512, 48*64) block and the 16
w-bins tile it exactly, so the per-core input is a flat (128, 98304)
array where partition p = (b_local*16 + iw) owns one contiguous w-bin
(32 rows x 3072 floats). The per-partition reduction keeps (ih=6, m=64)
-> 384 outputs = exactly the per-(b, iw) slice of the output. Both
input and output DMAs are perfectly coalesced, no transposes.

v2 pipeline (per core), after trace analysis of v1:
 - Loads alternate between the two HWDGE rings (SP via nc.sync, ACT via
   nc.scalar) into 3 rotating SBUF slots; 7 tiles of 4 w-rows (6.3 MB)
   + 2 tapered tiles of 2 rows (3.1 MB) to shrink the serial tail.
 - DVE folds w-rows with unit-stride tensor_tensor max chains
   (1.04 ns/elem; the strided 4D tensor_reduce of v1 ran at 1.61
   cyc/elem and made DVE the bottleneck at 170 us).
 - h-fold (8 -> 1 in bins of 8 cols) as a 3-step pairwise TT-max tree.
 - DVE total ~107 us < DMA ~148 us -> DMA continuously busy at the
   ~358 GB/s/core HBM fair share (measured 21.5 GB/s x 16 SDMA engines).
Raw Bass (not Tile): slot-reuse ordering lives in standalone sequencer
wait_ge instructions; Tile attaches 2 waits to the DMA instruction
itself, which overflows DMA_DIRECT2D's 1-wait budget in walrus codegen.
"""

import sys

sys.path.insert(0, "/opt/trn_rl_repo")

import numpy as np

import concourse.bass as bass
from concourse import mybir
from concourse.bass_utils import run_bass_kernel_spmd

N_CORES = 8
B, W, H, M = 64, 512, 48, 64
POOL_W, POOL_H = 16, 6
BIN_W, BIN_H = W // POOL_W, H // POOL_H  # 32, 8
B_LOC = B // N_CORES  # 8 samples per core
P = B_LOC * POOL_W  # 128 partitions = (b_local, iw)
ROW = H * M  # 3072 floats per w-row per partition
FREE = BIN_W * ROW  # 98304 elems per partition (one w-bin)
OUT_FREE = POOL_H * M  # 384
N_SLOTS = 3
SLOT_ROWS = 4
# 7 full tiles + 2 half tiles: taper the end so the serial tail
# (last TT chain + h-fold + out DMA) hides behind less DMA.
TILES = [(k * 4, 4) for k in range(7)] + [(28, 2), (30, 2)]

F32 = mybir.dt.float32

_cached = {}


def _build():
    if "nc" in _cached:
        return _cached["nc"]
    nc = bass.Bass()
    x = nc.dram_tensor("x", [P, FREE], F32, kind="ExternalInput")
    out = nc.dram_tensor("out", [P, OUT_FREE], F32, kind="ExternalOutput")

    with (
        nc.sbuf_tensor([P, N_SLOTS, SLOT_ROWS * ROW], F32) as slots,
        nc.sbuf_tensor([P, ROW], F32) as acc,
        nc.sbuf_tensor([P, POOL_H * 4 * M], F32) as tmp1,
        nc.sbuf_tensor([P, POOL_H * 2 * M], F32) as tmp2,
        nc.sbuf_tensor([P, OUT_FREE], F32) as res,
        nc.semaphore() as sp_sem,  # SP-ring load completions, +16 each
        nc.semaphore() as act_sem,  # ACT-ring load completions, +16 each
        nc.semaphore() as free_sem,  # DVE done consuming tile k, +1
        nc.semaphore() as res_sem,  # final result ready
        nc.semaphore() as out_sem,  # output DMA completion
        nc.Block() as block,
    ):
        # tile k issued on ring (k % 2): 0 -> SP, 1 -> ACT
        def emit_loads(eng, parity, sem):
            for k, (row0, nrows) in enumerate(TILES):
                if k % 2 != parity:
                    continue
                if k >= N_SLOTS:
                    eng.wait_ge(free_sem, k - N_SLOTS + 1)
                eng.dma_start(
                    out=slots[:, k % N_SLOTS, 0 : nrows * ROW],
                    in_=x[:, row0 * ROW : (row0 + nrows) * ROW],
                ).then_inc(sem, 16)

        @block.sync
        def _(s):
            emit_loads(s, 0, sp_sem)

        @block.scalar
        def _(sc):
            emit_loads(sc, 1, act_sem)
            sc.wait_ge(res_sem, 1)
            sc.dma_start(out=out[:, :], in_=res[:, :]).then_inc(out_sem, 16)
            sc.wait_ge(out_sem, 16)

        @block.vector
        def _(v):
            mx = mybir.AluOpType.max
            n_done = [0, 0]  # completed loads per ring

            for k, (row0, nrows) in enumerate(TILES):
                ring = k % 2
                n_done[ring] += 1
                v.wait_ge(sp_sem if ring == 0 else act_sem, 16 * n_done[ring])
                sl = slots[:, k % N_SLOTS, :]
                r0 = 0
                if k == 0:
                    # first op seeds acc from rows 0 and 1
                    v.tensor_tensor(
                        out=acc[:, :], in0=sl[:, 0:ROW], in1=sl[:, ROW : 2 * ROW], op=mx
                    )
                    r0 = 2
                for r in range(r0, nrows):
                    ins = v.tensor_tensor(
                        out=acc[:, :],
                        in0=acc[:, :],
                        in1=sl[:, r * ROW : (r + 1) * ROW],
                        op=mx,
                    )
                ins.then_inc(free_sem, 1)

            # h-fold: (ih, hh, m) max over hh as a pairwise tree
            def fold(dst, src, hh):
                a = src[:, :].rearrange("p (ih hh m) -> p ih hh m", ih=POOL_H, hh=hh, m=M)
                return v.tensor_tensor(
                    out=dst[:, :],
                    in0=a[:, :, 0 : hh // 2, :],
                    in1=a[:, :, hh // 2 : hh, :],
                    op=mx,
                )

            fold(tmp1, acc, BIN_H)
            fold(tmp2, tmp1, 4)
            fold(res, tmp2, 2).then_inc(res_sem, 1)

    _cached["nc"] = nc
    return nc


def kernel(x: np.ndarray, **run_kwargs) -> np.ndarray:
    nc = _build()
    x = np.ascontiguousarray(x, dtype=np.float32)
    xs = x.reshape(N_CORES, P, FREE)
    in_maps = [{"x": xs[c]} for c in range(N_CORES)]
    r = run_bass_kernel_spmd(nc, in_maps, core_ids=list(range(N_CORES)), **run_kwargs)
    out = np.concatenate(
        [r.results[c]["out"].reshape(B_LOC, POOL_W * OUT_FREE) for c in range(N_CORES)],
        axis=0,
    )
    if run_kwargs:
        return out, r
    return out


# revision 12
# speedup vs baseline: 1.1046x; 1.1046x over previous
"""Dimension-adaptive max pooling for sensors — Trainium2 Bass kernel.

Problem: x (64, 512, 48, 64) f32 -> out (64, 16*6*64) = (64, 6144) f32.
Adaptive max pool over spatial dims (512, 48) into (16, 6) bins. Since
512/16 = 32 and 48/6 = 8 exactly, each output bin is a plain max over a
(32, 8) window:

  out[b, iw*384 + ih*64 + m] = max_{r<32, hh<8} x[b, iw*32+r, ih*8+hh, m]

Sharding: pure data parallel over batch. 8 cores x 8 samples each.

Per-core layout: x[b] is a contiguous (512, 48*64) block and the 16
w-bins tile it exactly, so the per-core input is a flat (128, 98304)
array where partition p = (b_local*16 + iw) owns one contiguous w-bin
(32 rows x 3072 floats). The per-partition reduction keeps (ih=6, m=64)
-> 384 outputs = exactly the per-(b, iw) slice of the output. Both
input and output DMAs are perfectly coalesced, no transposes.

v5 pipeline (per core), after trace analysis of v1-v4:
 - 18 sub-loads on the single SP HWDGE ring (nc.sync): 14 x 2 w-rows
   (3.15 MB) + 4 x 1 row tapered at the end, into 6 rotating SBUF
   sub-slots. HWDGE sustains 26 GB/s/SDMA-engine vs SWDGE's 21.5.
 - DMA-completion semaphores can lead the actual SBUF data visibility
   by a small window (observed as stale-slot maxima under NTFF
   profiling, for both SWDGE and HWDGE). The DVE therefore reads tile j
   only after load j+1's semaphore (a full sub-load ~7.5 us of margin);
   a tiny chaser DMA after the last load closes the final tile. This
   made every traced run bitwise-exact.
 - DVE folds w-rows with unit-stride tensor_tensor max chains into TWO
   alternating accumulators: a single in-place chain ran at 1.26
   cyc/elem (dependent-op stall); alternating restores ~1.05.
 - h-fold (8 -> 1 cols): fold acc_a early (while waiting for the last
   row), then acc_b, pairwise-tree the rest; ACT-ring DMA (nc.scalar)
   writes the result so it does not queue behind loads.
Raw Bass (not Tile): slot-reuse ordering lives in standalone sequencer
wait_ge instructions; Tile attaches 2 waits to the DMA instruction
itself, which overflows DMA_DIRECT2D's 1-wait budget in walrus codegen.
"""

import sys

sys.path.insert(0, "/opt/trn_rl_repo")

import numpy as np

import concourse.bass as bass
from concourse import mybir
from concourse.bass_utils import run_bass_kernel_spmd

N_CORES = 8
B, W, H, M = 64, 512, 48, 64
POOL_W, POOL_H = 16, 6
BIN_W, BIN_H = W // POOL_W, H // POOL_H  # 32, 8
B_LOC = B // N_CORES  # 8 samples per core
P = B_LOC * POOL_W  # 128 partitions = (b_local, iw)
ROW = H * M  # 3072 floats per w-row per partition
FREE = BIN_W * ROW  # 98304 elems per partition (one w-bin)
OUT_FREE = POOL_H * M  # 384
N_SLOTS = 6
SLOT_ROWS = 2
# rows 0..29 in 2-row tiles feeding the accumulators, then row 30
# (last accumulator update), then row 31 which bypasses the
# accumulators entirely via its own fold path — so the critical chain
# after the last byte lands is just fold8+fold4+fold2+max, ~4 us.
TILES = [(k * 2, 2) for k in range(15)] + [(30, 1), (31, 1)]
NT = len(TILES)  # 17

F32 = mybir.dt.float32

_cached = {}


def _build():
    if "nc" in _cached:
        return _cached["nc"]
    nc = bass.Bass()
    x = nc.dram_tensor("x", [P, FREE], F32, kind="ExternalInput")
    out = nc.dram_tensor("out", [P, OUT_FREE], F32, kind="ExternalOutput")

    with (
        nc.sbuf_tensor([P, N_SLOTS, SLOT_ROWS * ROW], F32) as slots,
        nc.sbuf_tensor([P, 16], F32) as scratch,
        nc.sbuf_tensor([P, ROW], F32) as acc_a,
        nc.sbuf_tensor([P, ROW], F32) as acc_b,
        nc.sbuf_tensor([P, POOL_H * 4 * M], F32) as fa,
        nc.sbuf_tensor([P, POOL_H * 4 * M], F32) as fb,
        nc.sbuf_tensor([P, POOL_H * 2 * M], F32) as tmp2,
        nc.sbuf_tensor([P, OUT_FREE], F32) as res,
        nc.semaphore() as dma_sem,  # load completions, +16 each
        nc.semaphore() as free_sem,  # DVE done consuming tile j, +1
        nc.semaphore() as res_sem,  # final result ready
        nc.semaphore() as out_sem,  # output DMA completion
        nc.Block() as block,
    ):

        @block.gpsimd
        def _(g):
            # SWDGE, not HWDGE: each of the 16 SDMA engines increments the
            # sem by 1 after its own portion, so dma_sem >= 16*(k+2) proves
            # every engine finished load k+1 and (per-engine FIFO) load k's
            # data writes long retired. HWDGE's +16 does not bound
            # per-engine progress: traced runs kept reading stale slots
            # even with a full-load margin.
            for k, (row0, nrows) in enumerate(TILES):
                if k >= N_SLOTS:
                    g.wait_ge(free_sem, k - N_SLOTS + 1)
                g.dma_start(
                    out=slots[:, k % N_SLOTS, 0 : nrows * ROW],
                    in_=x[:, row0 * ROW : (row0 + nrows) * ROW],
                ).then_inc(dma_sem, 16)
            # chaser: tiny ring-ordered load so the last tile also gets a
            # +1-load completion margin
            g.dma_start(out=scratch[:, :], in_=x[:, 0:16]).then_inc(dma_sem, 16)

        @block.scalar
        def _(sc):
            sc.wait_ge(res_sem, 1)
            sc.dma_start(out=out[:, :], in_=res[:, :]).then_inc(out_sem, 16)
            sc.wait_ge(out_sem, 16)

        @block.vector
        def _(v):
            mx = mybir.AluOpType.max

            def row(sl, r):
                return sl[:, r * ROW : (r + 1) * ROW]

            def fold(dst, src, hh, src_is_ap=False):
                a = (src if src_is_ap else src[:, :]).rearrange(
                    "p (ih hh m) -> p ih hh m", ih=POOL_H, hh=hh, m=M
                )
                return v.tensor_tensor(
                    out=dst[:, :],
                    in0=a[:, :, 0 : hh // 2, :],
                    in1=a[:, :, hh // 2 : hh, :],
                    op=mx,
                )

            for k, (row0, nrows) in enumerate(TILES[:-1]):
                # margin: tile j readable once load j+1 (or the chaser)
                # has completed
                v.wait_ge(dma_sem, 16 * (k + 2))
                sl = slots[:, k % N_SLOTS, :]
                if k == 0:
                    ins = v.tensor_tensor(
                        out=acc_a[:, :], in0=row(sl, 0), in1=row(sl, 1), op=mx
                    )
                elif k == 1:
                    ins = v.tensor_tensor(
                        out=acc_b[:, :], in0=row(sl, 0), in1=row(sl, 1), op=mx
                    )
                else:
                    for r in range(nrows):
                        acc = acc_a if ((row0 + r) % 2 == 0) else acc_b
                        ins = v.tensor_tensor(
                            out=acc[:, :], in0=acc[:, :], in1=row(sl, r), op=mx
                        )
                ins.then_inc(free_sem, 1)
                if k == NT - 3:
                    # acc_b's final update was row 29 (tile NT-3); fold it
                    # while waiting for the row-30 load
                    fold(fb, acc_b, BIN_H)

            # acc_a complete (row 30 was tile NT-2): fold the accumulator
            # tree down to 384 while the row-31 load + chaser complete
            fold(fa, acc_a, BIN_H)
            v.tensor_tensor(out=fa[:, :], in0=fa[:, :], in1=fb[:, :], op=mx)
            fold(tmp2, fa, 4)
            fold(res, tmp2, 2)

            # row 31 bypasses the accumulators: fold it directly and merge
            k = NT - 1
            v.wait_ge(dma_sem, 16 * (k + 2))  # chaser closes the margin
            sl = slots[:, k % N_SLOTS, :]
            fold(fb, sl[:, 0:ROW], BIN_H, src_is_ap=True)
            fold(tmp2, fb, 4)
            fold(fb[:, 0:OUT_FREE], tmp2, 2)
            v.tensor_tensor(
                out=res[:, :], in0=res[:, :], in1=fb[:, 0:OUT_FREE], op=mx
            ).then_inc(res_sem, 1)

    _cached["nc"] = nc
    return nc


def kernel(x: np.ndarray, **run_kwargs) -> np.ndarray:
    nc = _build()
    x = np.ascontiguousarray(x, dtype=np.float32)
    xs = x.reshape(N_CORES, P, FREE)
    in_maps = [{"x": xs[c]} for c in range(N_CORES)]
    r = run_bass_kernel_spmd(nc, in_maps, core_ids=list(range(N_CORES)), **run_kwargs)
    out = np.concatenate(
        [r.results[c]["out"].reshape(B_LOC, POOL_W * OUT_FREE) for c in range(N_CORES)],
        axis=0,
    )
    if run_kwargs:
        return out, r
    return out
